# revision 12
# baseline (speedup 1.0000x reference)
"""Single-head attention kernel for Trainium2, SPMD over 8 NeuronCores.

Problem: x [4,4096,128], Wq/Wk/Wv [128,128] -> y [4,4096,128]
  q = x @ Wq.T ; k = x @ Wk.T ; v = x @ Wv.T
  y = softmax(q k^T / sqrt(128)) v

Sharding: 8 cores = 4 batches x 2 query-halves. Each core receives its
batch's x rotated so that its 2048 queries are rows 0..2047 (attention is
invariant to permuting the key order, so rotation changes nothing) -> all
cores run the identical NEFF with no dynamic offsets and no collectives.

Per-core dataflow (all attention matmuls bf16 inputs, f32 PSUM accum):
  xT: cast-DMA x to bf16 in DRAM (SWDGE), then XBAR transpose-DMA into
      SBUF chunks [128h, 512n] -- zero compute-engine cycles spent.
  kT = Wk @ xT chunks, qT = Wq @ xT[:2048], v = x @ Wv^T (PE, bf16)
  for each 1024-query block:
    for each of 32 key tiles:
      S^T = kT-tile^T @ qT-block     (PE, 2x N=512 into [128k,1024] PSUM)
      A^T = exp(S^T * scale)         (ACT, one op per 1024, bf16 SBUF)
      yT += v-tile^T @ A^T           (PE, [128o,1024q] PSUM accum)
      3-level bf16 pair-tree of A^T  (DVE, softmax denominator)
    l  = ones^T @ tree-roots         (PE accumulating [1,512]x2 PSUM)
    y  = transpose(yT) * (1/l)       (PE bf16 transpose + DVE per-part scale)
"""

import sys

sys.path.insert(0, "/opt/trn_rl_repo")

import numpy as np

import concourse.bass as bass
import concourse.mybir as mybir
from concourse import bacc
from concourse.bass_utils import run_bass_kernel_spmd
from concourse.tile import TileContext
from concourse.masks import make_identity

P = 128
N = 4096  # context length (per batch)
NQ = 2048  # queries per core
H = 128
O = 128
KT = N // P  # 32 key tiles
NC = N // 512  # 8 column chunks of 512
QBS = 1024  # query block size
QB = NQ // QBS  # 2 query blocks
SCALE = 1.0 / np.sqrt(128.0)

F32 = mybir.dt.float32
BF16 = mybir.dt.bfloat16

_cached_nc = None


def build_kernel():
    nc = bacc.Bacc(None, target_bir_lowering=False)

    x_d = nc.declare_dram_parameter("x", [N, H], F32, isOutput=False)
    w_d = {
        "q": nc.declare_dram_parameter("wq", [H, H], F32, isOutput=False),
        "k": nc.declare_dram_parameter("wk", [H, H], F32, isOutput=False),
        "v": nc.declare_dram_parameter("wv", [O, H], F32, isOutput=False),
    }
    out_d = nc.declare_dram_parameter("out", [NQ, O], F32, isOutput=True)

    with TileContext(nc) as tc:
        with (
            tc.tile_pool(name="const", bufs=1) as cpool,
            tc.tile_pool(name="big", bufs=1) as big,
            tc.tile_pool(name="dram", bufs=1, space="DRAM") as dpool,
        ):
            ident_bf = cpool.tile([P, P], BF16)
            make_identity(nc, ident_bf)
            ident_f32 = cpool.tile([1, 1], F32)
            nc.gpsimd.memset(ident_f32[:], 1.0)
            ones_bf = cpool.tile([P, 1], BF16)
            nc.gpsimd.memset(ones_bf[:], 1.0)

            # persistent bf16 operand chunks [128, 512]
            xTs = [big.tile([P, 512], BF16, name=f"xT{c}") for c in range(NC)]
            kTs = [big.tile([P, 512], BF16, name=f"kT{c}") for c in range(NC)]
            qTs = [big.tile([P, 512], BF16, name=f"qT{c}") for c in range(NQ // 512)]
            vss = [big.tile([P, 512], BF16, name=f"vs{c}") for c in range(NC)]
            wT = {
                "q": big.tile([P, P], BF16, name="wqT"),
                "k": big.tile([P, P], BF16, name="wkT"),
                "v": big.tile([P, P], BF16, name="wvT"),
            }

            def kslice(tiles, kt):
                return tiles[kt // 4][:, (kt % 4) * P : (kt % 4 + 1) * P]

            # ---- Stage A: x -> bf16 (cast DMA) -> xT chunks (XBAR transpose)
            for name in ("q", "k", "v"):
                wbf = dpool.tile([H, H], BF16, name=f"wbf_{name}")
                nc.gpsimd.dma_start(out=wbf[:], in_=w_d[name][:])  # cast
                nc.sync.dma_start(out=wT[name][:], in_=wbf[:], transpose=True)
            for c in range(NC):
                xbf = dpool.tile([512, H], BF16, name=f"xbf{c}")
                nc.gpsimd.dma_start(
                    out=xbf[:], in_=x_d[c * 512 : (c + 1) * 512, :]
                )  # f32 -> bf16 cast, DRAM -> DRAM
                nc.sync.dma_start(out=xTs[c][:], in_=xbf[:], transpose=True)

            # ---- Stage C: projections (kT/qT copies on ACT, v copies on DVE)
            with tc.tile_pool(name="psc", bufs=4, space="PSUM") as psc:
                for c in range(NC):
                    pk = psc.tile([P, 512], F32, tag="pk")
                    nc.tensor.matmul(
                        pk[:], wT["k"][:], xTs[c][:], start=True, stop=True
                    )
                    nc.scalar.activation(
                        kTs[c][:], pk[:], mybir.ActivationFunctionType.Copy
                    )
                for c in range(NQ // 512):
                    pq = psc.tile([P, 512], F32, tag="pk")
                    nc.tensor.matmul(
                        pq[:], wT["q"][:], xTs[c][:], start=True, stop=True
                    )
                    nc.scalar.activation(
                        qTs[c][:], pq[:], mybir.ActivationFunctionType.Copy
                    )
                for c in range(NC):
                    pv = psc.tile([P, 512], F32, tag="pk")
                    for t4 in range(4):
                        nc.tensor.matmul(
                            pv[:, t4 * P : (t4 + 1) * P],
                            xTs[c][:, t4 * P : (t4 + 1) * P],
                            wT["v"][:],
                            start=True, stop=True,
                        )
                    nc.vector.tensor_copy(vss[c][:], pv[:])

            # ---- Stage D: attention, per query block
            with (
                tc.tile_pool(name="ps_s", bufs=2, space="PSUM") as ps_s,
                tc.tile_pool(name="ps_y", bufs=1, space="PSUM") as ps_y,
                tc.tile_pool(name="ps_sm", bufs=2, space="PSUM") as ps_sm,
                tc.tile_pool(name="apool", bufs=4) as apool,
                tc.tile_pool(name="tpool", bufs=3) as tpool,
                tc.tile_pool(name="epi", bufs=2) as epi,
            ):
                for qb in range(QB):
                    py = ps_y.tile([P, QBS], F32, tag="py")

                    # software-pipelined S/exp emission so PE runs ahead of ACT
                    DEPTH = 2
                    a_tiles = {}
                    tree_prev = {1: None, 2: None}
                    roots = []

                    def emit_s_exp(kt):
                        ps = ps_s.tile([P, QBS], F32, tag="ps")
                        for h in range(QBS // 512):
                            nc.tensor.matmul(
                                ps[:, h * 512 : (h + 1) * 512],
                                kslice(kTs, kt),
                                qTs[qb * 2 + h][:],
                                start=True, stop=True,
                            )
                        a = apool.tile([P, QBS], BF16, tag="a")
                        nc.scalar.activation(
                            a[:], ps[:], mybir.ActivationFunctionType.Exp,
                            scale=float(SCALE),
                        )
                        a_tiles[kt] = a

                    def tree_add(level, t):
                        out = tpool.tile([P, QBS], BF16, tag=f"t{level}")
                        prev = tree_prev[level]
                        nc.vector.tensor_tensor(
                            out[:], prev[:], t[:], mybir.AluOpType.add
                        )
                        tree_prev[level] = None
                        return out

                    for kt in range(DEPTH):
                        emit_s_exp(kt)
                    for kt in range(KT):
                        if kt + DEPTH < KT:
                            emit_s_exp(kt + DEPTH)
                        a = a_tiles.pop(kt)
                        for h in range(QBS // 512):
                            nc.tensor.matmul(
                                py[:, h * 512 : (h + 1) * 512],
                                kslice(vss, kt),
                                a[:, h * 512 : (h + 1) * 512],
                                start=(kt == 0), stop=(kt == KT - 1),
                            )
                        # 3-level pair tree for the softmax denominator
                        if tree_prev[1] is None:
                            tree_prev[1] = a
                        else:
                            p1 = tree_add(1, a)
                            if tree_prev[2] is None:
                                tree_prev[2] = p1
                            else:
                                roots.append(tree_add(2, p1))

                    # l = sum over k of A^T: accumulate ones^T @ root tiles
                    pls = [
                        ps_sm.tile([1, 512], F32, tag="sm", name=f"pl{_h}")
                        for _h in range(QBS // 512)
                    ]
                    nroots = len(roots)
                    for ri, r in enumerate(roots):
                        for h in range(QBS // 512):
                            nc.tensor.matmul(
                                pls[h][:],
                                ones_bf[:],
                                r[:, h * 512 : (h + 1) * 512],
                                start=(ri == 0), stop=(ri == nroots - 1),
                            )
                    l_row = epi.tile([1, QBS], F32, tag="l_row")
                    for h in range(QBS // 512):
                        nc.scalar.activation(
                            l_row[:, h * 512 : (h + 1) * 512], pls[h][:],
                            mybir.ActivationFunctionType.Copy,
                        )

                    y_t = epi.tile([P, QBS], BF16, tag="y_t")
                    nc.scalar.activation(
                        y_t[:], py[:], mybir.ActivationFunctionType.Copy
                    )

                    for j in range(QBS // P):
                        psm_l = ps_sm.tile([P, 1], F32, tag="sm")
                        nc.tensor.transpose(
                            psm_l[:], l_row[:, j * P : (j + 1) * P],
                            ident_f32[:],
                        )
                        psm_y = ps_sm.tile([P, P], BF16, tag="sm")
                        nc.tensor.transpose(
                            psm_y[:], y_t[:, j * P : (j + 1) * P], ident_bf[:]
                        )
                        lcol = epi.tile([P, 1], F32, tag="lcol")
                        nc.vector.reciprocal(lcol[:], psm_l[:])
                        yout = epi.tile([P, P], F32, tag="yout")
                        nc.vector.tensor_scalar_mul(
                            yout[:], psm_y[:], lcol[:, 0:1]
                        )
                        r0 = qb * QBS + j * P
                        nc.sync.dma_start(out=out_d[r0 : r0 + P, :], in_=yout[:])

    nc.compile()
    return nc


def _run(x, Wq, Wk, Wv, **spmd_kwargs):
    global _cached_nc
    if _cached_nc is None:
        _cached_nc = build_kernel()
    nc = _cached_nc

    x = np.asarray(x, dtype=np.float32)
    Wq = np.ascontiguousarray(np.asarray(Wq, dtype=np.float32))
    Wk = np.ascontiguousarray(np.asarray(Wk, dtype=np.float32))
    Wv = np.ascontiguousarray(np.asarray(Wv, dtype=np.float32))

    B = x.shape[0]
    in_maps = []
    for core in range(8):
        b, half = core // 2, core % 2
        xb = x[b]
        if half:
            xb = np.roll(xb, -NQ, axis=0)  # queries -> rows 0..NQ-1
        in_maps.append(
            {"x": np.ascontiguousarray(xb), "wq": Wq, "wk": Wk, "wv": Wv}
        )

    res = run_bass_kernel_spmd(nc, in_maps, core_ids=list(range(8)), **spmd_kwargs)

    y = np.empty((B, N, O), dtype=np.float32)
    for core in range(8):
        b, half = core // 2, core % 2
        y[b, half * NQ : (half + 1) * NQ] = res.results[core]["out"]
    return y, res


def kernel(x, Wq, Wk, Wv):
    y, _ = _run(x, Wq, Wk, Wv)
    return y


if __name__ == "__main__":
    rng = np.random.default_rng(0)
    x = rng.standard_normal((4, N, H), dtype=np.float32)
    Wq = rng.standard_normal((H, H), dtype=np.float32) / np.sqrt(H)
    Wk = rng.standard_normal((H, H), dtype=np.float32) / np.sqrt(H)
    Wv = rng.standard_normal((O, H), dtype=np.float32) / np.sqrt(H)
    y = kernel(x=x, Wq=Wq, Wk=Wk, Wv=Wv)
    print("kernel output", y.shape, y.dtype)


# revision 15
# speedup vs baseline: 1.3439x; 1.3439x over previous
"""Single-head attention kernel for Trainium2, SPMD over 8 NeuronCores.

Problem: x [4,4096,128], Wq/Wk/Wv [128,128] -> y [4,4096,128]
  q = x @ Wq.T ; k = x @ Wk.T ; v = x @ Wv.T
  y = softmax(q k^T / sqrt(128)) v

Sharding: 8 cores = 4 batches x 2 query-halves. Each core receives its
batch's x rotated so that its 2048 queries are rows 0..2047 (attention is
invariant to permuting the key order, so rotation changes nothing) -> all
cores run the identical NEFF with no dynamic offsets and no collectives.

Per-core dataflow (all attention matmuls bf16 inputs, f32 PSUM accum):
  xT: cast-DMA x to bf16 in DRAM (SWDGE), then XBAR transpose-DMA into
      SBUF chunks [128h, 512n] -- zero compute-engine cycles spent.
  kT = Wk @ xT chunks, qT = Wq @ xT[:2048], v = x @ Wv^T (PE, bf16)
  for each 1024-query block:
    for each of 32 key tiles:
      S^T = kT-tile^T @ qT-block     (PE, 2x N=512 into [128k,1024] PSUM)
      A^T = exp(S^T * scale)         (ACT, one op per 1024, bf16 SBUF)
      yT += v-tile^T @ A^T           (PE, [128o,1024q] PSUM accum)
      3-level bf16 pair-tree of A^T  (DVE, softmax denominator)
    l  = ones^T @ tree-roots         (PE accumulating [1,512]x2 PSUM)
    y  = transpose(yT) * (1/l)       (PE bf16 transpose + DVE per-part scale)
"""

import sys

sys.path.insert(0, "/opt/trn_rl_repo")

import numpy as np

import concourse.bass as bass
import concourse.mybir as mybir
from concourse import bacc
from concourse.bass_utils import run_bass_kernel_spmd
from concourse.tile import TileContext
from concourse.masks import make_identity

P = 128
N = 4096  # context length (per batch)
NQ = 2048  # queries per core
H = 128
O = 128
KT = N // P  # 32 key tiles
NC = N // 512  # 8 column chunks of 512
QBS = 1024  # query block size
QB = NQ // QBS  # 2 query blocks
SCALE = 1.0 / np.sqrt(128.0)

F32 = mybir.dt.float32
BF16 = mybir.dt.bfloat16

_cached_nc = None


def build_kernel():
    nc = bacc.Bacc(None, target_bir_lowering=False)

    x_d = nc.declare_dram_parameter("x", [N, H], F32, isOutput=False)
    w_d = {
        "q": nc.declare_dram_parameter("wq", [H, H], F32, isOutput=False),
        "k": nc.declare_dram_parameter("wk", [H, H], F32, isOutput=False),
        "v": nc.declare_dram_parameter("wv", [O, H], F32, isOutput=False),
    }
    out_d = nc.declare_dram_parameter("out", [NQ, O], F32, isOutput=True)

    with TileContext(nc) as tc:
        with (
            tc.tile_pool(name="const", bufs=1) as cpool,
            tc.tile_pool(name="big", bufs=1) as big,
        ):
            ident_bf = cpool.tile([P, P], BF16)
            make_identity(nc, ident_bf)
            ident_f32 = cpool.tile([1, 1], F32)
            nc.gpsimd.memset(ident_f32[:], 1.0)
            ident_f32x = cpool.tile([P, P], F32)
            make_identity(nc, ident_f32x)
            ones_bf = cpool.tile([P, 1], BF16)
            nc.gpsimd.memset(ones_bf[:], 1.0)

            # persistent bf16 operand chunks [128, 512]
            xTs = [big.tile([P, 512], BF16, name=f"xT{c}") for c in range(NC)]
            kTs = [big.tile([P, 512], BF16, name=f"kT{c}") for c in range(NC)]
            qTs = [big.tile([P, 512], BF16, name=f"qT{c}") for c in range(NQ // 512)]
            vss = [big.tile([P, 512], BF16, name=f"vs{c}") for c in range(NC)]
            wT = {
                "q": big.tile([P, P], BF16, name="wqT"),
                "k": big.tile([P, P], BF16, name="wkT"),
                "v": big.tile([P, P], BF16, name="wvT"),
            }

            def kslice(tiles, kt):
                return tiles[kt // 4][:, (kt % 4) * P : (kt % 4 + 1) * P]

            # ---- Stage A+C: per 512-row chunk: load f32, transpose on PE
            # (f32, 2c/row), cast to bf16 during the PSUM->SBUF copy, then
            # run this chunk's projections immediately (fine-grained deps
            # let stage D start once chunks 0-1 are projected).
            with (
                tc.tile_pool(name="stagea", bufs=3) as sta,
                tc.tile_pool(name="psa", bufs=3, space="PSUM") as psa,
                tc.tile_pool(name="psc", bufs=4, space="PSUM") as psc,
            ):
                # W loads (cast DMA) + PE transposes
                pw = psc.tile([P, 3 * P], BF16, tag="pk")
                for wi, name in enumerate(("q", "k", "v")):
                    wst = sta.tile([P, P], BF16, tag="wst")
                    nc.gpsimd.dma_start(out=wst[:], in_=w_d[name][:])  # cast
                    nc.tensor.transpose(
                        pw[:, wi * P : (wi + 1) * P], wst[:], ident_bf[:]
                    )
                for wi, name in enumerate(("q", "k", "v")):
                    nc.vector.tensor_copy(
                        wT[name][:], pw[:, wi * P : (wi + 1) * P]
                    )

                for c in range(NC):
                    xst = sta.tile([P, 4, P], F32, tag="xst")
                    rows = x_d[c * 512 : (c + 1) * 512, :]
                    nc.sync.dma_start(
                        out=xst[:], in_=rows.rearrange("(t p) h -> p t h", p=P)
                    )
                    px = psa.tile([P, 512], F32, tag="px")
                    for t4 in range(4):
                        nc.tensor.transpose(
                            px[:, t4 * P : (t4 + 1) * P], xst[:, t4, :],
                            ident_f32x[:],
                        )
                    nc.vector.tensor_copy(xTs[c][:], px[:])  # f32 -> bf16

                    pk = psc.tile([P, 512], F32, tag="pk")
                    nc.tensor.matmul(
                        pk[:], wT["k"][:], xTs[c][:], start=True, stop=True
                    )
                    nc.scalar.activation(
                        kTs[c][:], pk[:], mybir.ActivationFunctionType.Copy
                    )
                    if c < NQ // 512:
                        pq = psc.tile([P, 512], F32, tag="pk")
                        nc.tensor.matmul(
                            pq[:], wT["q"][:], xTs[c][:], start=True, stop=True
                        )
                        nc.scalar.activation(
                            qTs[c][:], pq[:], mybir.ActivationFunctionType.Copy
                        )
                    pv = psc.tile([P, 512], F32, tag="pk")
                    for t4 in range(4):
                        nc.tensor.matmul(
                            pv[:, t4 * P : (t4 + 1) * P],
                            xTs[c][:, t4 * P : (t4 + 1) * P],
                            wT["v"][:],
                            start=True, stop=True,
                        )
                    nc.vector.tensor_copy(vss[c][:], pv[:])

            # ---- Stage D: attention, per query block
            with (
                tc.tile_pool(name="ps_s", bufs=2, space="PSUM") as ps_s,
                tc.tile_pool(name="ps_y", bufs=1, space="PSUM") as ps_y,
                tc.tile_pool(name="ps_sm", bufs=2, space="PSUM") as ps_sm,
                tc.tile_pool(name="apool", bufs=4) as apool,
                tc.tile_pool(name="tpool", bufs=3) as tpool,
                tc.tile_pool(name="epi", bufs=2) as epi,
            ):
                for qb in range(QB):
                    py = ps_y.tile([P, QBS], F32, tag="py")

                    # software-pipelined S/exp emission so PE runs ahead of ACT
                    DEPTH = 2
                    a_tiles = {}
                    tree_prev = {1: None, 2: None}
                    roots = []

                    def emit_s_exp(kt):
                        ps = ps_s.tile([P, QBS], F32, tag="ps")
                        for h in range(QBS // 512):
                            nc.tensor.matmul(
                                ps[:, h * 512 : (h + 1) * 512],
                                kslice(kTs, kt),
                                qTs[qb * 2 + h][:],
                                start=True, stop=True,
                            )
                        a = apool.tile([P, QBS], BF16, tag="a")
                        nc.scalar.activation(
                            a[:], ps[:], mybir.ActivationFunctionType.Exp,
                            scale=float(SCALE),
                        )
                        a_tiles[kt] = a

                    def tree_add(level, t):
                        out = tpool.tile([P, QBS], BF16, tag=f"t{level}")
                        prev = tree_prev[level]
                        nc.vector.tensor_tensor(
                            out[:], prev[:], t[:], mybir.AluOpType.add
                        )
                        tree_prev[level] = None
                        return out

                    for kt in range(DEPTH):
                        emit_s_exp(kt)
                    for kt in range(KT):
                        if kt + DEPTH < KT:
                            emit_s_exp(kt + DEPTH)
                        a = a_tiles.pop(kt)
                        for h in range(QBS // 512):
                            nc.tensor.matmul(
                                py[:, h * 512 : (h + 1) * 512],
                                kslice(vss, kt),
                                a[:, h * 512 : (h + 1) * 512],
                                start=(kt == 0), stop=(kt == KT - 1),
                            )
                        # 3-level pair tree for the softmax denominator
                        if tree_prev[1] is None:
                            tree_prev[1] = a
                        else:
                            p1 = tree_add(1, a)
                            if tree_prev[2] is None:
                                tree_prev[2] = p1
                            else:
                                roots.append(tree_add(2, p1))

                    # l = sum over k of A^T: accumulate ones^T @ root tiles
                    pls = [
                        ps_sm.tile([1, 512], F32, tag="sm", name=f"pl{_h}")
                        for _h in range(QBS // 512)
                    ]
                    nroots = len(roots)
                    for ri, r in enumerate(roots):
                        for h in range(QBS // 512):
                            nc.tensor.matmul(
                                pls[h][:],
                                ones_bf[:],
                                r[:, h * 512 : (h + 1) * 512],
                                start=(ri == 0), stop=(ri == nroots - 1),
                            )
                    l_row = epi.tile([1, QBS], F32, tag="l_row")
                    for h in range(QBS // 512):
                        nc.scalar.activation(
                            l_row[:, h * 512 : (h + 1) * 512], pls[h][:],
                            mybir.ActivationFunctionType.Copy,
                        )

                    y_t = epi.tile([P, QBS], BF16, tag="y_t")
                    nc.scalar.activation(
                        y_t[:], py[:], mybir.ActivationFunctionType.Copy
                    )

                    for j in range(QBS // P):
                        psm_l = ps_sm.tile([P, 1], F32, tag="sm")
                        nc.tensor.transpose(
                            psm_l[:], l_row[:, j * P : (j + 1) * P],
                            ident_f32[:],
                        )
                        psm_y = ps_sm.tile([P, P], BF16, tag="sm")
                        nc.tensor.transpose(
                            psm_y[:], y_t[:, j * P : (j + 1) * P], ident_bf[:]
                        )
                        lcol = epi.tile([P, 1], F32, tag="lcol")
                        nc.vector.reciprocal(lcol[:], psm_l[:])
                        yout = epi.tile([P, P], F32, tag="yout")
                        nc.vector.tensor_scalar_mul(
                            yout[:], psm_y[:], lcol[:, 0:1]
                        )
                        r0 = qb * QBS + j * P
                        nc.sync.dma_start(out=out_d[r0 : r0 + P, :], in_=yout[:])

    nc.compile()
    return nc


def _run(x, Wq, Wk, Wv, **spmd_kwargs):
    global _cached_nc
    if _cached_nc is None:
        _cached_nc = build_kernel()
    nc = _cached_nc

    x = np.asarray(x, dtype=np.float32)
    Wq = np.ascontiguousarray(np.asarray(Wq, dtype=np.float32))
    Wk = np.ascontiguousarray(np.asarray(Wk, dtype=np.float32))
    Wv = np.ascontiguousarray(np.asarray(Wv, dtype=np.float32))

    B = x.shape[0]
    in_maps = []
    for core in range(8):
        b, half = core // 2, core % 2
        xb = x[b]
        if half:
            xb = np.roll(xb, -NQ, axis=0)  # queries -> rows 0..NQ-1
        in_maps.append(
            {"x": np.ascontiguousarray(xb), "wq": Wq, "wk": Wk, "wv": Wv}
        )

    res = run_bass_kernel_spmd(nc, in_maps, core_ids=list(range(8)), **spmd_kwargs)

    y = np.empty((B, N, O), dtype=np.float32)
    for core in range(8):
        b, half = core // 2, core % 2
        y[b, half * NQ : (half + 1) * NQ] = res.results[core]["out"]
    return y, res


def kernel(x, Wq, Wk, Wv):
    y, _ = _run(x, Wq, Wk, Wv)
    return y


if __name__ == "__main__":
    rng = np.random.default_rng(0)
    x = rng.standard_normal((4, N, H), dtype=np.float32)
    Wq = rng.standard_normal((H, H), dtype=np.float32) / np.sqrt(H)
    Wk = rng.standard_normal((H, H), dtype=np.float32) / np.sqrt(H)
    Wv = rng.standard_normal((O, H), dtype=np.float32) / np.sqrt(H)
    y = kernel(x=x, Wq=Wq, Wk=Wk, Wv=Wv)
    print("kernel output", y.shape, y.dtype)
